# revision 3
# baseline (speedup 1.0000x reference)
"""Multi-head self-attention (B=4, T=2048, C=1024, 16 heads x hd=64) on 8
Trainium2 NeuronCores.

Sharding: tensor-parallel over heads — each core owns 2 heads (128 of the
1024 channels): its slices of Wq/Wk/Wv rows and Wo columns. Every core reads
the full x (transposed + bf16-cast on host), computes Q^T/K^T (channel-major)
and V (token-major) for its heads, runs attention entirely from SBUF, then
produces a rank-128 partial of the output projection. The 8 partials are
summed on host (+ bo).

Per-core dataflow (all matmuls bf16 in / fp32 PSUM accumulate):
  phase 1: Q^T = Wq_c @ x^T (+bq), K^T = Wk_c @ x^T (bk dropped — it only
           shifts every score in a softmax row by a constant), V = x @ Wv_c^T
           token-major with a ones column appended per head (denominator
           trick). All SBUF-resident.
  phase 2: per (batch, 512-query block): S^T tiles [128k, 512q] via
           K^T-stationary matmuls (two heads packed in the 128x128 PE array
           via tile_position row groups, contraction d=64), exp on ScalarE
           (scale=1/8 folded in) -> P^T bf16, then O^T[65,512] = [V|1]^T P^T
           accumulated over k (two k-parities packed via row groups into two
           PSUM banks). Merge banks, reciprocal of the ones-row, broadcast
           via a zero-step SBUF->SBUF DMA, normalize, +bv.
  phase 3: partial_out[128 rows, 512] = O^T-slice-stationary matmuls against
           Wo_c^T; fp16 partials DMA'd out.
"""
import json

import numpy as np
import ml_dtypes

import concourse.bass as bass
import concourse.mybir as mybir
import concourse.tile as tile
from concourse.bass_utils import run_bass_kernel_spmd

bf16 = ml_dtypes.bfloat16
dt = mybir.dt

EMB = 1024
HEADS = 16
HD = 64
B = 4
T = 2048
R = B * T            # 8192 rows
NCORES = 8
F = EMB // NCORES    # 128 channels (2 heads) per core
NH = F // HD         # 2 heads per core
NKC = EMB // 128     # 8 contraction chunks for projections
NRB = R // 512       # 16 row blocks of 512
NQB = T // 512       # 4 query blocks per batch
NKT = T // 128       # 16 key tiles per batch
G = R // 128         # 64 global row/key tiles
VW = HD + 1          # 65: V head slice + ones column


# ---------------------------------------------------------------------------
# walrus in this container accepts only ONE sync-wait per instruction; split
# extra waits onto same-engine NoOps at BIR-serialization time.
_orig_to_json_bytes = bass.Bass.to_json_bytes


def _split_waits(data: bytes) -> bytes:
    d = json.loads(data)
    changed = False
    for f in d.get("functions", []):
        for blk in f.get("blocks", []):
            out = []
            for inst in blk.get("instructions", []):
                si = inst.get("sync_info")
                waits = (si or {}).get("on_wait") or []
                if len(waits) > 1:
                    changed = True
                    for i, w in enumerate(waits[:-1]):
                        out.append({
                            "debug": inst.get("debug", 0),
                            "engine": inst["engine"],
                            "ins": [], "outs": [],
                            "name": f"{inst['name']}_w{i}",
                            "opcode": "NoOp",
                            "sync_info": {"on_update": [], "on_wait": [w]},
                            "text_hint": "wait_split",
                        })
                    si["on_wait"] = waits[-1:]
                out.append(inst)
            blk["instructions"] = out
    return json.dumps(d).encode() if changed else data


def _to_json_bytes(self, *a, **k):
    return _split_waits(_orig_to_json_bytes(self, *a, **k))


bass.Bass.to_json_bytes = _to_json_bytes
# ---------------------------------------------------------------------------


def build_bass() -> bass.Bass:
    nc = bass.Bass()
    xt_ext = nc.declare_dram_parameter("xt", [EMB, R], dt.bfloat16, isOutput=False)
    wq_ext = nc.declare_dram_parameter("wq", [EMB, F], dt.bfloat16, isOutput=False)
    wk_ext = nc.declare_dram_parameter("wk", [EMB, F], dt.bfloat16, isOutput=False)
    wv_ext = nc.declare_dram_parameter("wv", [EMB, F], dt.bfloat16, isOutput=False)
    wo_ext = nc.declare_dram_parameter("wo", [F, EMB], dt.bfloat16, isOutput=False)
    bq_ext = nc.declare_dram_parameter("bq", [F, 1], dt.float32, isOutput=False)
    bv_ext = nc.declare_dram_parameter("bv", [F, 1], dt.float32, isOutput=False)
    out_ext = nc.declare_dram_parameter("out", [R, EMB], dt.float16, isOutput=True)

    Exp = mybir.ActivationFunctionType.Exp

    with tile.TileContext(nc) as tc:
        with (
            tc.tile_pool(name="const", bufs=1) as cp,
            tc.tile_pool(name="res", bufs=1) as res,
            tc.tile_pool(name="xt", bufs=2) as xp,
            tc.tile_pool(name="pt", bufs=4) as ptp,
            tc.tile_pool(name="norm", bufs=2) as np_,
            tc.tile_pool(name="osb", bufs=3) as op,
            tc.tile_pool(name="ps", bufs=1, space="PSUM") as ps,
        ):
            # --- constants ---
            wq_sb = cp.tile([128, EMB], dt.bfloat16, tag="wq")
            wk_sb = cp.tile([128, EMB], dt.bfloat16, tag="wk")
            wv_sb = cp.tile([128, EMB], dt.bfloat16, tag="wv")
            wo_sb = cp.tile([128, EMB], dt.bfloat16, tag="wo")
            bq_sb = cp.tile([F, 1], dt.float32, tag="bq")
            bv_sb = cp.tile([F, 1], dt.float32, tag="bv")
            for ext, tile_sb in ((wq_ext, wq_sb), (wk_ext, wk_sb), (wv_ext, wv_sb)):
                nc.sync.dma_start(
                    tile_sb[:].rearrange("p (kc f) -> p kc f", f=F),
                    ext[:].rearrange("(kc p) f -> p kc f", p=128),
                )
            nc.sync.dma_start(wo_sb[:], wo_ext[:])
            nc.sync.dma_start(bq_sb[:], bq_ext[:])
            nc.sync.dma_start(bv_sb[:], bv_ext[:])

            # --- residents ---
            qt_sb = res.tile([F, R], dt.bfloat16, tag="qt")
            kt_sb = res.tile([F, R], dt.bfloat16, tag="kt")
            ot_sb = res.tile([F, R], dt.bfloat16, tag="ot")
            va_sb = res.tile([128, G * NH * VW], dt.bfloat16, tag="va")
            # ones columns (denominator trick), one strided memset
            nc.vector.memset(
                va_sb[:].rearrange("p (g d) -> p g d", d=VW)[:, :, HD:VW], 1.0
            )

            # ---------------- phase 1: projections ----------------
            for rb in range(NRB):
                r0 = rb * 512
                xts = []
                for kc in range(NKC):
                    xt = xp.tile([128, 512], dt.bfloat16, tag=f"xt{kc}")
                    nc.sync.dma_start(
                        xt[:], xt_ext[kc * 128:(kc + 1) * 128, r0:r0 + 512]
                    )
                    xts.append(xt)
                # Q^T block [F, 512]
                q_ps = ps.tile([128, 512], dt.float32, tag="proj", bufs=2)
                for kc in range(NKC):
                    nc.tensor.matmul(
                        q_ps[:], wq_sb[:, kc * F:(kc + 1) * F], xts[kc][:],
                        start=(kc == 0), stop=(kc == NKC - 1),
                    )
                nc.vector.tensor_scalar_add(
                    qt_sb[:, r0:r0 + 512], q_ps[:], bq_sb[:]
                )
                # K^T block [F, 512] (no bias)
                k_ps = ps.tile([128, 512], dt.float32, tag="proj", bufs=2)
                for kc in range(NKC):
                    nc.tensor.matmul(
                        k_ps[:], wk_sb[:, kc * F:(kc + 1) * F], xts[kc][:],
                        start=(kc == 0), stop=(kc == NKC - 1),
                    )
                nc.vector.tensor_copy(kt_sb[:, r0:r0 + 512], k_ps[:])
                # V token-major: 4 sub-tiles of 128 rows
                for sub in range(4):
                    g = rb * 4 + sub
                    v_ps = ps.tile([128, F], dt.float32, tag="proj", bufs=2)
                    for kc in range(NKC):
                        nc.tensor.matmul(
                            v_ps[:],
                            xts[kc][:, sub * 128:(sub + 1) * 128],
                            wv_sb[:, kc * F:(kc + 1) * F],
                            start=(kc == 0), stop=(kc == NKC - 1),
                        )
                    dst = va_sb[:, g * NH * VW:(g + 1) * NH * VW].rearrange(
                        "p (h d) -> p h d", d=VW
                    )[:, :, 0:HD]
                    nc.vector.tensor_copy(
                        dst, v_ps[:].rearrange("p (h d) -> p h d", d=HD)
                    )

            # ---------------- phase 2: attention ----------------
            for b in range(B):
                for qb in range(NQB):
                    q0 = b * T + qb * 512
                    pv = {}
                    for h in range(NH):
                        pv_lo = ps.tile([VW, 512], dt.float32, tag="pv", bufs=4,
                                        name=f"pv_lo_{b}_{qb}_{h}")
                        pv_hi = ps.tile([VW, 512], dt.float32, tag="pv", bufs=4,
                                        name=f"pv_hi_{b}_{qb}_{h}")
                        pv[h] = (pv_lo, pv_hi)
                    for j in range(NKT):
                        k0 = b * T + j * 128
                        g = b * NKT + j
                        pts = []
                        for h in range(NH):
                            st = ps.tile([128, 512], dt.float32, tag="st", bufs=2)
                            nc.tensor.matmul(
                                st[:],
                                kt_sb[h * HD:(h + 1) * HD, k0:k0 + 128],
                                qt_sb[h * HD:(h + 1) * HD, q0:q0 + 512],
                                start=True, stop=True,
                                tile_position=(h * HD, 0),
                            )
                            pt = ptp.tile([128, 512], dt.bfloat16, tag="pt")
                            nc.scalar.activation(pt[:], st[:], Exp, scale=0.125)
                            pts.append(pt)
                        for h in range(NH):
                            va = va_sb[:, g * NH * VW + h * VW:
                                       g * NH * VW + (h + 1) * VW]
                            nc.tensor.matmul(
                                pv[h][0][:], va[0:64, :], pts[h][0:64, :],
                                start=(j == 0), stop=(j == NKT - 1),
                                tile_position=(0, 0),
                            )
                            nc.tensor.matmul(
                                pv[h][1][:], va[64:128, :], pts[h][64:128, :],
                                start=(j == 0), stop=(j == NKT - 1),
                                tile_position=(64, 0),
                            )
                    for h in range(NH):
                        m = np_.tile([VW, 512], dt.float32, tag="m")
                        nc.vector.tensor_copy(m[:], pv[h][0][:])
                        nc.vector.tensor_add(m[:], m[:], pv[h][1][:])
                        rc = np_.tile([1, 512], dt.float32, tag="rc")
                        nc.vector.reciprocal(rc[:], m[HD:VW, :])
                        rbt = np_.tile([HD, 512], dt.float32, tag="rb")
                        nc.sync.dma_start(
                            rbt[:],
                            rc[0:1, :].rearrange("p (o q) -> p o q", o=1)
                            .broadcast_to((1, HD, 512)),
                        )
                        osl = ot_sb[h * HD:(h + 1) * HD, q0:q0 + 512]
                        nc.vector.tensor_mul(osl, m[0:HD, :], rbt[:])
                        nc.vector.tensor_scalar_add(
                            osl, osl, bv_sb[h * HD:(h + 1) * HD, :]
                        )

            # ---------------- phase 3: output projection ----------------
            for g in range(G):
                for ch in range(2):
                    o_ps = ps.tile([128, 512], dt.float32, tag="proj", bufs=2)
                    nc.tensor.matmul(
                        o_ps[:],
                        ot_sb[:, g * 128:(g + 1) * 128],
                        wo_sb[:, ch * 512:(ch + 1) * 512],
                        start=True, stop=True,
                    )
                    o_sb = op.tile([128, 512], dt.float16, tag="osb")
                    nc.vector.tensor_copy(o_sb[:], o_ps[:])
                    nc.sync.dma_start(
                        out_ext[g * 128:(g + 1) * 128, ch * 512:(ch + 1) * 512],
                        o_sb[:],
                    )
    return nc


_NC_CACHE = None


def _get_nc():
    global _NC_CACHE
    if _NC_CACHE is None:
        _NC_CACHE = build_bass()
    return _NC_CACHE


def make_in_maps(x, Wq, bq, Wk, bk, Wv, bv, Wo, bo):
    xt = np.ascontiguousarray(
        np.asarray(x, dtype=np.float32).reshape(R, EMB).astype(bf16).T
    )
    in_maps = []
    for c in range(NCORES):
        rows = slice(F * c, F * (c + 1))
        in_maps.append({
            "xt": xt,
            "wq": np.ascontiguousarray(np.asarray(Wq)[rows, :].T.astype(bf16)),
            "wk": np.ascontiguousarray(np.asarray(Wk)[rows, :].T.astype(bf16)),
            "wv": np.ascontiguousarray(np.asarray(Wv)[rows, :].T.astype(bf16)),
            "wo": np.ascontiguousarray(np.asarray(Wo)[:, rows].T.astype(bf16)),
            "bq": np.asarray(bq)[rows].reshape(F, 1).astype(np.float32),
            "bv": np.asarray(bv)[rows].reshape(F, 1).astype(np.float32),
        })
    return in_maps


def gather(results, bo):
    acc = np.zeros((R, EMB), np.float32)
    for r in results:
        acc += r["out"].astype(np.float32)
    acc += np.asarray(bo, dtype=np.float32)
    return acc.reshape(B, T, EMB)


def kernel(x, Wq, bq, Wk, bk, Wv, bv, Wo, bo, _trace=False):
    nc = _get_nc()
    in_maps = make_in_maps(x, Wq, bq, Wk, bk, Wv, bv, Wo, bo)
    res = run_bass_kernel_spmd(nc, in_maps, list(range(NCORES)), trace=_trace)
    out = gather(res.results, bo)
    if _trace:
        kernel.last_result = res
    return out


# revision 5
# speedup vs baseline: 1.0556x; 1.0556x over previous
"""Multi-head self-attention (B=4, T=2048, C=1024, 16 heads x hd=64) on 8
Trainium2 NeuronCores.

Sharding: tensor-parallel over heads — each core owns 2 heads (128 of the
1024 channels): its slices of Wq/Wk/Wv rows and Wo columns. Every core reads
the full x (transposed + bf16-cast on host), computes Q^T/K^T (channel-major)
and V (token-major) for its heads, runs attention entirely from SBUF, then
produces a rank-128 partial of the output projection. The 8 partials are
summed on host (+ bo).

Per-core dataflow (all matmuls bf16 in / fp32 PSUM accumulate):
  phase 1: Q^T = Wq_c @ x^T (+bq), K^T = Wk_c @ x^T (bk dropped — it only
           shifts every score in a softmax row by a constant), V = x @ Wv_c^T
           token-major with a ones column appended per head (denominator
           trick). All SBUF-resident.
  phase 2: per (batch, 512-query block): S^T [128k, 1024(2 k-tiles)] per head
           via K^T-stationary matmuls (contraction d=64), one exp per k-tile
           pair on ScalarE (scale=1/8 folded in) -> P^T bf16, then
           O^T[65,512] += [V|1]^T P^T accumulated over k with K=128 matmuls.
           Softmax denominator lands in row 64; its reciprocal is computed
           128-lanes-wide by DMA-reshaping [1,512]->[128,4], then DMA'd back
           and broadcast with a zero-step DMA; normalize + bv on VectorE.
  phase 3: partial_out[128 rows, 512] = O^T-slice-stationary matmuls against
           Wo_c^T; fp16 partials DMA'd out.

Phases 1 and 3 are emitted as filler groups inside phase 2's ACT-bound loops
(batch b+1 projections and batch b-1 output projections run in the PE gaps
while ScalarE chews exponentials), and PV consumption is software-pipelined
one tile-pair behind S^T production.
"""
import json

import numpy as np
import ml_dtypes

import concourse.bass as bass
import concourse.mybir as mybir
import concourse.tile as tile
from concourse.bass_utils import run_bass_kernel_spmd

bf16 = ml_dtypes.bfloat16
dt = mybir.dt

EMB = 1024
HEADS = 16
HD = 64
B = 4
T = 2048
R = B * T            # 8192 rows
NCORES = 8
F = EMB // NCORES    # 128 channels (2 heads) per core
NH = F // HD         # 2 heads per core
NKC = EMB // 128     # 8 contraction chunks for projections
NQB = T // 512       # 4 query blocks per batch
NJP = T // 256       # 8 k-tile PAIRS per batch
G = R // 128         # 64 global row/key tiles
VW = HD + 1          # 65: V head slice + ones column


# ---------------------------------------------------------------------------
# walrus in this container accepts only ONE sync-wait per instruction; split
# extra waits onto same-engine NoOps at BIR-serialization time.
_orig_to_json_bytes = bass.Bass.to_json_bytes


def _split_waits(data: bytes) -> bytes:
    d = json.loads(data)
    changed = False
    for f in d.get("functions", []):
        for blk in f.get("blocks", []):
            out = []
            for inst in blk.get("instructions", []):
                si = inst.get("sync_info")
                waits = (si or {}).get("on_wait") or []
                if len(waits) > 1:
                    changed = True
                    for i, w in enumerate(waits[:-1]):
                        out.append({
                            "debug": inst.get("debug", 0),
                            "engine": inst["engine"],
                            "ins": [], "outs": [],
                            "name": f"{inst['name']}_w{i}",
                            "opcode": "NoOp",
                            "sync_info": {"on_update": [], "on_wait": [w]},
                            "text_hint": "wait_split",
                        })
                    si["on_wait"] = waits[-1:]
                out.append(inst)
            blk["instructions"] = out
    return json.dumps(d).encode() if changed else data


def _to_json_bytes(self, *a, **k):
    return _split_waits(_orig_to_json_bytes(self, *a, **k))


bass.Bass.to_json_bytes = _to_json_bytes
# ---------------------------------------------------------------------------


def build_bass() -> bass.Bass:
    nc = bass.Bass()
    xt_ext = nc.declare_dram_parameter("xt", [EMB, R], dt.bfloat16, isOutput=False)
    wq_ext = nc.declare_dram_parameter("wq", [EMB, F], dt.bfloat16, isOutput=False)
    wk_ext = nc.declare_dram_parameter("wk", [EMB, F], dt.bfloat16, isOutput=False)
    wv_ext = nc.declare_dram_parameter("wv", [EMB, F], dt.bfloat16, isOutput=False)
    wo_ext = nc.declare_dram_parameter("wo", [F, EMB], dt.bfloat16, isOutput=False)
    bq_ext = nc.declare_dram_parameter("bq", [F, 1], dt.float32, isOutput=False)
    bv_ext = nc.declare_dram_parameter("bv", [F, 1], dt.float32, isOutput=False)
    out_ext = nc.declare_dram_parameter("out", [R, EMB], dt.float16, isOutput=True)

    Exp = mybir.ActivationFunctionType.Exp

    with tile.TileContext(nc) as tc:
        with (
            tc.tile_pool(name="const", bufs=1) as cp,
            tc.tile_pool(name="res", bufs=1) as res,
            tc.tile_pool(name="xt", bufs=2) as xp,
            tc.tile_pool(name="pt", bufs=4) as ptp,
            tc.tile_pool(name="norm", bufs=2) as npl,
            tc.tile_pool(name="osb", bufs=3) as op,
            tc.tile_pool(name="ps", bufs=1, space="PSUM") as ps,
        ):
            # --- constants ---
            wq_sb = cp.tile([128, EMB], dt.bfloat16, tag="wq")
            wk_sb = cp.tile([128, EMB], dt.bfloat16, tag="wk")
            wv_sb = cp.tile([128, EMB], dt.bfloat16, tag="wv")
            wo_sb = cp.tile([128, EMB], dt.bfloat16, tag="wo")
            bq_sb = cp.tile([F, 1], dt.float32, tag="bq")
            bv_sb = cp.tile([F, 1], dt.float32, tag="bv")
            for ext, tile_sb in ((wq_ext, wq_sb), (wk_ext, wk_sb), (wv_ext, wv_sb)):
                nc.sync.dma_start(
                    tile_sb[:].rearrange("p (kc f) -> p kc f", f=F),
                    ext[:].rearrange("(kc p) f -> p kc f", p=128),
                )
            nc.sync.dma_start(wo_sb[:], wo_ext[:])
            nc.sync.dma_start(bq_sb[:], bq_ext[:])
            nc.sync.dma_start(bv_sb[:], bv_ext[:])

            # --- residents ---
            qt_sb = res.tile([F, R], dt.bfloat16, tag="qt")
            kt_sb = res.tile([F, R], dt.bfloat16, tag="kt")
            ot_sb = res.tile([F, R], dt.bfloat16, tag="ot")
            va_sb = res.tile([128, G * NH * VW], dt.bfloat16, tag="va")
            nc.vector.memset(
                va_sb[:].rearrange("p (g d) -> p g d", d=VW)[:, :, HD:VW], 1.0
            )

            # ---- phase-1 emitters (one 512-row block = 6 filler groups) ----
            def p1_load(rb):
                xts = []
                for kc in range(NKC):
                    xt = xp.tile([128, 512], dt.bfloat16, tag=f"xt{kc}",
                                 name=f"xt{kc}_{rb}")
                    nc.sync.dma_start(
                        xt[:],
                        xt_ext[kc * 128:(kc + 1) * 128, rb * 512:rb * 512 + 512],
                    )
                    xts.append(xt)
                return xts

            def p1_qk(rb, xts, w_sb, dst_sb, bias):
                r0 = rb * 512
                acc = ps.tile([128, 512], dt.float32, tag="pp", bufs=2,
                              name=f"prj_{rb}_{id(w_sb)}")
                for kc in range(NKC):
                    nc.tensor.matmul(
                        acc[:], w_sb[:, kc * F:(kc + 1) * F], xts[kc][:],
                        start=(kc == 0), stop=(kc == NKC - 1),
                    )
                if bias is not None:
                    nc.vector.tensor_scalar_add(dst_sb[:, r0:r0 + 512], acc[:], bias[:])
                else:
                    nc.vector.tensor_copy(dst_sb[:, r0:r0 + 512], acc[:])

            def p1_v(rb, xts, sub):
                g = rb * 4 + sub
                acc = ps.tile([128, F], dt.float32, tag="pp", bufs=2,
                              name=f"vprj_{g}")
                for kc in range(NKC):
                    nc.tensor.matmul(
                        acc[:],
                        xts[kc][:, sub * 128:(sub + 1) * 128],
                        wv_sb[:, kc * F:(kc + 1) * F],
                        start=(kc == 0), stop=(kc == NKC - 1),
                    )
                dst = va_sb[:, g * NH * VW:(g + 1) * NH * VW].rearrange(
                    "p (h d) -> p h d", d=VW
                )[:, :, 0:HD]
                nc.vector.tensor_copy(
                    dst, acc[:].rearrange("p (h d) -> p h d", d=HD)
                )

            def p1_block_fillers(rb):
                """6 filler closures for one 512-row projection block."""
                state = {}

                def load_and_q():
                    state["xts"] = p1_load(rb)
                    p1_qk(rb, state["xts"], wq_sb, qt_sb, bq_sb)

                fillers = [load_and_q,
                           lambda: p1_qk(rb, state["xts"], wk_sb, kt_sb, None)]
                for sub in range(4):
                    fillers.append(lambda s=sub: p1_v(rb, state["xts"], s))
                return fillers

            # ---- phase-3 emitter (one 128-row tile) ----
            def p3_tile(g):
                o_sb = op.tile([128, EMB], dt.float16, tag="osb", name=f"o_{g}")
                for ch in range(2):
                    o_ps = ps.tile([128, 512], dt.float32, tag="pp", bufs=2,
                                   name=f"ops_{g}_{ch}")
                    nc.tensor.matmul(
                        o_ps[:],
                        ot_sb[:, g * 128:(g + 1) * 128],
                        wo_sb[:, ch * 512:(ch + 1) * 512],
                        start=True, stop=True,
                    )
                    nc.vector.tensor_copy(o_sb[:, ch * 512:(ch + 1) * 512], o_ps[:])
                nc.sync.dma_start(out_ext[g * 128:(g + 1) * 128, :], o_sb[:])

            # ---- phase-2 q-block with interleaved fillers ----
            def p2_qblock(b, qb, fillers):
                q0 = b * T + qb * 512
                fi = iter(fillers)

                def fill(n=1):
                    for _ in range(n):
                        f = next(fi, None)
                        if f is not None:
                            f()

                pvs = {h: ps.tile([VW, 512], dt.float32, tag="pv", bufs=2,
                                  name=f"pv_{b}_{qb}_{h}")
                       for h in range(NH)}
                pts = {}

                def emit_st(jp):
                    k0 = b * T + jp * 256
                    for h in range(NH):
                        st = ps.tile([128, 1024], dt.float32, tag="st", bufs=2,
                                     name=f"st_{b}_{qb}_{jp}_{h}")
                        for half in range(2):
                            nc.tensor.matmul(
                                st[:, half * 512:(half + 1) * 512],
                                kt_sb[h * HD:(h + 1) * HD,
                                      k0 + half * 128:k0 + (half + 1) * 128],
                                qt_sb[h * HD:(h + 1) * HD, q0:q0 + 512],
                                start=True, stop=True,
                            )
                        pt = ptp.tile([128, 1024], dt.bfloat16, tag="pt",
                                      name=f"pt_{b}_{qb}_{jp}_{h}")
                        nc.scalar.activation(pt[:], st[:], Exp, scale=0.125)
                        pts[(jp, h)] = pt

                def emit_pv(jp):
                    g0 = b * NJP * 2 + jp * 2
                    for h in range(NH):
                        pt = pts.pop((jp, h))
                        for half in range(2):
                            g = g0 + half
                            va = va_sb[:, g * NH * VW + h * VW:
                                       g * NH * VW + (h + 1) * VW]
                            nc.tensor.matmul(
                                pvs[h][:], va[:],
                                pt[:, half * 512:(half + 1) * 512],
                                start=(jp == 0 and half == 0),
                                stop=(jp == NJP - 1 and half == 1),
                            )

                for jp in range(NJP):
                    emit_st(jp)
                    if jp > 0:
                        emit_pv(jp - 1)
                    fill(1)
                emit_pv(NJP - 1)
                fill(2)
                # normalize both heads
                for h in range(NH):
                    pv = pvs[h]
                    dc = npl.tile([1, 512], dt.float32, tag="dc", name=f"dc_{b}_{qb}_{h}")
                    nc.vector.tensor_copy(dc[:], pv[HD:VW, :])
                    d4 = npl.tile([128, 4], dt.float32, tag="d4", name=f"d4_{b}_{qb}_{h}")
                    nc.sync.dma_start(
                        d4[:],
                        dc[:].rearrange("p (a c) -> p a c", c=4),
                    )
                    r4 = npl.tile([128, 4], dt.float32, tag="r4", name=f"r4_{b}_{qb}_{h}")
                    nc.vector.reciprocal(r4[:], d4[:])
                    rc = npl.tile([1, 512], dt.float32, tag="rc", name=f"rc_{b}_{qb}_{h}")
                    nc.sync.dma_start(
                        rc[:].rearrange("p (a c) -> p a c", c=4), r4[:]
                    )
                    rbt = npl.tile([HD, 512], dt.float32, tag="rb", name=f"rb_{b}_{qb}_{h}")
                    nc.sync.dma_start(
                        rbt[:],
                        rc[0:1, :].rearrange("p (o q) -> p o q", o=1)
                        .broadcast_to((1, HD, 512)),
                    )
                    osl = ot_sb[h * HD:(h + 1) * HD, q0:q0 + 512]
                    nc.vector.tensor_mul(osl, pv[0:HD, :], rbt[:])
                    nc.vector.tensor_scalar_add(
                        osl, osl, bv_sb[h * HD:(h + 1) * HD, :]
                    )
                fill(100)   # drain any leftover fillers

            # ---------------- emission schedule ----------------
            # batch 0 projections upfront
            for rb in range(4):
                for f in p1_block_fillers(rb):
                    f()
            for b in range(B):
                for qb in range(NQB):
                    fillers = []
                    if b < B - 1:
                        fillers.extend(p1_block_fillers((b + 1) * 4 + qb))
                    if b > 0:
                        for g in range((b - 1) * 16 + qb * 4,
                                       (b - 1) * 16 + qb * 4 + 4):
                            fillers.append(lambda g=g: p3_tile(g))
                    p2_qblock(b, qb, fillers)
            # tail: output projection for batch 3
            for g in range(48, 64):
                p3_tile(g)
    return nc


_NC_CACHE = None


def _get_nc():
    global _NC_CACHE
    if _NC_CACHE is None:
        _NC_CACHE = build_bass()
    return _NC_CACHE


def make_in_maps(x, Wq, bq, Wk, bk, Wv, bv, Wo, bo):
    xt = np.ascontiguousarray(
        np.asarray(x, dtype=np.float32).reshape(R, EMB).astype(bf16).T
    )
    in_maps = []
    for c in range(NCORES):
        rows = slice(F * c, F * (c + 1))
        in_maps.append({
            "xt": xt,
            "wq": np.ascontiguousarray(np.asarray(Wq)[rows, :].T.astype(bf16)),
            "wk": np.ascontiguousarray(np.asarray(Wk)[rows, :].T.astype(bf16)),
            "wv": np.ascontiguousarray(np.asarray(Wv)[rows, :].T.astype(bf16)),
            "wo": np.ascontiguousarray(np.asarray(Wo)[:, rows].T.astype(bf16)),
            "bq": np.asarray(bq)[rows].reshape(F, 1).astype(np.float32),
            "bv": np.asarray(bv)[rows].reshape(F, 1).astype(np.float32),
        })
    return in_maps


def gather(results, bo):
    acc = np.zeros((R, EMB), np.float32)
    for r in results:
        acc += r["out"].astype(np.float32)
    acc += np.asarray(bo, dtype=np.float32)
    return acc.reshape(B, T, EMB)


def kernel(x, Wq, bq, Wk, bk, Wv, bv, Wo, bo, _trace=False):
    nc = _get_nc()
    in_maps = make_in_maps(x, Wq, bq, Wk, bk, Wv, bv, Wo, bo)
    res = run_bass_kernel_spmd(nc, in_maps, list(range(NCORES)), trace=_trace)
    out = gather(res.results, bo)
    if _trace:
        kernel.last_result = res
    return out


# revision 7
# speedup vs baseline: 1.4725x; 1.3949x over previous
"""Multi-head self-attention (B=4, T=2048, C=1024, 16 heads x hd=64) on 8
Trainium2 NeuronCores.

Sharding: tensor-parallel over heads — each core owns 2 heads (128 of the
1024 channels): its slices of Wq/Wk/Wv rows and Wo columns. Every core reads
the full x (transposed + bf16-cast on host), computes Q^T/K^T (channel-major)
and V (token-major) for its heads, runs attention entirely from SBUF, then
produces a rank-128 partial of the output projection. The 8 partials are
summed on host (+ bo).

Per-core dataflow (all matmuls bf16 in / fp32 PSUM accumulate):
  phase 1: Q^T = Wq_c @ x^T (+bq), K^T = Wk_c @ x^T (bk dropped — it only
           shifts every score in a softmax row by a constant), V = x @ Wv_c^T
           token-major with a ones column appended per head (denominator
           trick). All SBUF-resident.
  phase 2: per (batch, 512-query block): S^T [128k, 1024(2 k-tiles)] per head
           via K^T-stationary matmuls (contraction d=64), one exp per k-tile
           pair on ScalarE (scale=1/8 folded in) -> P^T bf16, then
           O^T[65,512] += [V|1]^T P^T accumulated over k with K=128 matmuls.
           Softmax denominator lands in row 64; its reciprocal is computed
           128-lanes-wide by DMA-reshaping [1,512]->[128,4], then DMA'd back
           and broadcast with a zero-step DMA; normalize + bv on VectorE.
  phase 3: partial_out[128 rows, 512] = O^T-slice-stationary matmuls against
           Wo_c^T; fp16 partials DMA'd out.

Phases 1 and 3 are emitted as filler groups inside phase 2's ACT-bound loops
(batch b+1 projections and batch b-1 output projections run in the PE gaps
while ScalarE chews exponentials), and PV consumption is software-pipelined
one tile-pair behind S^T production.
"""
import json

import numpy as np
import ml_dtypes

import concourse.bass as bass
import concourse.mybir as mybir
import concourse.tile as tile
from concourse.bass_utils import run_bass_kernel_spmd

bf16 = ml_dtypes.bfloat16
dt = mybir.dt

EMB = 1024
HEADS = 16
HD = 64
B = 4
T = 2048
R = B * T            # 8192 rows
NCORES = 8
F = EMB // NCORES    # 128 channels (2 heads) per core
NH = F // HD         # 2 heads per core
NKC = EMB // 128     # 8 contraction chunks for projections
NQB = T // 512       # 4 query blocks per batch
NJP = T // 256       # 8 k-tile PAIRS per batch
G = R // 128         # 64 global row/key tiles
VW = HD + 1          # 65: V head slice + ones column


# ---------------------------------------------------------------------------
# walrus in this container accepts only ONE sync-wait per instruction; split
# extra waits onto same-engine NoOps at BIR-serialization time.
_orig_to_json_bytes = bass.Bass.to_json_bytes


def _split_waits(data: bytes) -> bytes:
    d = json.loads(data)
    changed = False
    for f in d.get("functions", []):
        for blk in f.get("blocks", []):
            out = []
            for inst in blk.get("instructions", []):
                si = inst.get("sync_info")
                waits = (si or {}).get("on_wait") or []
                if len(waits) > 1:
                    changed = True
                    for i, w in enumerate(waits[:-1]):
                        out.append({
                            "debug": inst.get("debug", 0),
                            "engine": inst["engine"],
                            "ins": [], "outs": [],
                            "name": f"{inst['name']}_w{i}",
                            "opcode": "NoOp",
                            "sync_info": {"on_update": [], "on_wait": [w]},
                            "text_hint": "wait_split",
                        })
                    si["on_wait"] = waits[-1:]
                out.append(inst)
            blk["instructions"] = out
    return json.dumps(d).encode() if changed else data


def _to_json_bytes(self, *a, **k):
    return _split_waits(_orig_to_json_bytes(self, *a, **k))


bass.Bass.to_json_bytes = _to_json_bytes
# ---------------------------------------------------------------------------


def build_bass() -> bass.Bass:
    nc = bass.Bass()
    xt_ext = nc.declare_dram_parameter("xt", [EMB, R], dt.bfloat16, isOutput=False)
    wq_ext = nc.declare_dram_parameter("wq", [EMB, F], dt.bfloat16, isOutput=False)
    wk_ext = nc.declare_dram_parameter("wk", [EMB, F], dt.bfloat16, isOutput=False)
    wv_ext = nc.declare_dram_parameter("wv", [EMB, F], dt.bfloat16, isOutput=False)
    wo_ext = nc.declare_dram_parameter("wo", [F, EMB], dt.bfloat16, isOutput=False)
    bq_ext = nc.declare_dram_parameter("bq", [F, 1], dt.float32, isOutput=False)
    bv_ext = nc.declare_dram_parameter("bv", [F, 1], dt.float32, isOutput=False)
    out_ext = nc.declare_dram_parameter("out", [R, EMB], dt.float16, isOutput=True)

    Exp = mybir.ActivationFunctionType.Exp

    with tile.TileContext(nc) as tc:
        with (
            tc.tile_pool(name="const", bufs=1) as cp,
            tc.tile_pool(name="res", bufs=1) as res,
            tc.tile_pool(name="xt", bufs=2) as xp,
            tc.tile_pool(name="pt", bufs=4) as ptp,
            tc.tile_pool(name="norm", bufs=2) as npl,
            tc.tile_pool(name="osb", bufs=3) as op,
            tc.tile_pool(name="ps", bufs=1, space="PSUM") as ps,
        ):
            # --- constants ---
            wq_sb = cp.tile([128, EMB], dt.bfloat16, tag="wq")
            wk_sb = cp.tile([128, EMB], dt.bfloat16, tag="wk")
            wv_sb = cp.tile([128, EMB], dt.bfloat16, tag="wv")
            wo_sb = cp.tile([128, EMB], dt.bfloat16, tag="wo")
            bq_sb = cp.tile([F, 1], dt.float32, tag="bq")
            bv_sb = cp.tile([F, 1], dt.float32, tag="bv")
            for ext, tile_sb in ((wq_ext, wq_sb), (wk_ext, wk_sb), (wv_ext, wv_sb)):
                nc.sync.dma_start(
                    tile_sb[:].rearrange("p (kc f) -> p kc f", f=F),
                    ext[:].rearrange("(kc p) f -> p kc f", p=128),
                )
            nc.sync.dma_start(wo_sb[:], wo_ext[:])
            nc.sync.dma_start(bq_sb[:], bq_ext[:])
            nc.sync.dma_start(bv_sb[:], bv_ext[:])

            # --- residents ---
            qt_sb = res.tile([F, R], dt.bfloat16, tag="qt")
            kt_sb = res.tile([F, R], dt.bfloat16, tag="kt")
            ot_sb = res.tile([F, R], dt.bfloat16, tag="ot")
            va_sb = res.tile([128, G * NH * VW], dt.bfloat16, tag="va")
            nc.vector.memset(
                va_sb[:].rearrange("p (g d) -> p g d", d=VW)[:, :, HD:VW], 1.0
            )

            # ---- phase-1 emitters (one 512-row block = 6 filler groups) ----
            def p1_load(rb):
                xts = []
                for kc in range(NKC):
                    xt = xp.tile([128, 512], dt.bfloat16, tag=f"xt{kc}",
                                 name=f"xt{kc}_{rb}")
                    nc.sync.dma_start(
                        xt[:],
                        xt_ext[kc * 128:(kc + 1) * 128, rb * 512:rb * 512 + 512],
                    )
                    xts.append(xt)
                return xts

            def p1_qk(rb, xts, w_sb, dst_sb, bias):
                r0 = rb * 512
                acc = ps.tile([128, 512], dt.float32, tag="pp", bufs=2,
                              name=f"prj_{rb}_{id(w_sb)}")
                for kc in range(NKC):
                    nc.tensor.matmul(
                        acc[:], w_sb[:, kc * F:(kc + 1) * F], xts[kc][:],
                        start=(kc == 0), stop=(kc == NKC - 1),
                    )
                if bias is not None:
                    nc.vector.tensor_scalar_add(dst_sb[:, r0:r0 + 512], acc[:], bias[:])
                else:
                    nc.vector.tensor_copy(dst_sb[:, r0:r0 + 512], acc[:])

            def p1_v(rb, xts, sub):
                g = rb * 4 + sub
                acc = ps.tile([128, F], dt.float32, tag="pp", bufs=2,
                              name=f"vprj_{g}")
                for kc in range(NKC):
                    nc.tensor.matmul(
                        acc[:],
                        xts[kc][:, sub * 128:(sub + 1) * 128],
                        wv_sb[:, kc * F:(kc + 1) * F],
                        start=(kc == 0), stop=(kc == NKC - 1),
                    )
                dst = va_sb[:, g * NH * VW:(g + 1) * NH * VW].rearrange(
                    "p (h d) -> p h d", d=VW
                )[:, :, 0:HD]
                nc.vector.tensor_copy(
                    dst, acc[:].rearrange("p (h d) -> p h d", d=HD)
                )

            def p1_block_fillers(rb):
                """7 filler closures for one 512-row projection block; the
                DMA prefetch is its own filler so compute fillers that
                follow a few slots later never wait on it."""
                state = {}

                def load():
                    state["xts"] = p1_load(rb)

                fillers = [load,
                           lambda: p1_qk(rb, state["xts"], wq_sb, qt_sb, bq_sb),
                           lambda: p1_qk(rb, state["xts"], wk_sb, kt_sb, None)]
                for sub in range(4):
                    fillers.append(lambda s=sub: p1_v(rb, state["xts"], s))
                return fillers

            # ---- phase-3 emitter (one 128-row tile) ----
            def p3_tile(g):
                o_sb = op.tile([128, EMB], dt.float16, tag="osb", name=f"o_{g}")
                for ch in range(2):
                    o_ps = ps.tile([128, 512], dt.float32, tag="pp", bufs=2,
                                   name=f"ops_{g}_{ch}")
                    nc.tensor.matmul(
                        o_ps[:],
                        ot_sb[:, g * 128:(g + 1) * 128],
                        wo_sb[:, ch * 512:(ch + 1) * 512],
                        start=True, stop=True,
                    )
                    nc.vector.tensor_copy(o_sb[:, ch * 512:(ch + 1) * 512], o_ps[:])
                nc.sync.dma_start(out_ext[g * 128:(g + 1) * 128, :], o_sb[:])

            # ---- phase-2 q-block with interleaved fillers ----
            def p2_qblock(b, qb, fillers):
                q0 = b * T + qb * 512
                fi = iter(fillers)

                def fill(n=1):
                    for _ in range(n):
                        f = next(fi, None)
                        if f is not None:
                            f()

                pvs = {h: ps.tile([VW, 512], dt.float32, tag="pv", bufs=2,
                                  name=f"pv_{b}_{qb}_{h}")
                       for h in range(NH)}
                pts = {}

                def emit_st(jp):
                    k0 = b * T + jp * 256
                    for h in range(NH):
                        st = ps.tile([128, 1024], dt.float32, tag="st", bufs=2,
                                     name=f"st_{b}_{qb}_{jp}_{h}")
                        for half in range(2):
                            nc.tensor.matmul(
                                st[:, half * 512:(half + 1) * 512],
                                kt_sb[h * HD:(h + 1) * HD,
                                      k0 + half * 128:k0 + (half + 1) * 128],
                                qt_sb[h * HD:(h + 1) * HD, q0:q0 + 512],
                                start=True, stop=True,
                            )
                        pt = ptp.tile([128, 1024], dt.bfloat16, tag="pt",
                                      name=f"pt_{b}_{qb}_{jp}_{h}")
                        nc.scalar.activation(pt[:], st[:], Exp, scale=0.125)
                        pts[(jp, h)] = pt

                def emit_pv(jp):
                    g0 = b * NJP * 2 + jp * 2
                    for h in range(NH):
                        pt = pts.pop((jp, h))
                        for half in range(2):
                            g = g0 + half
                            va = va_sb[:, g * NH * VW + h * VW:
                                       g * NH * VW + (h + 1) * VW]
                            nc.tensor.matmul(
                                pvs[h][:], va[:],
                                pt[:, half * 512:(half + 1) * 512],
                                start=(jp == 0 and half == 0),
                                stop=(jp == NJP - 1 and half == 1),
                            )

                for jp in range(NJP):
                    emit_st(jp)
                    if jp > 0:
                        emit_pv(jp - 1)
                    fill(1)
                emit_pv(NJP - 1)
                fill(2)
                # normalize both heads; the [65,512] SBUF copy releases the
                # PSUM slot immediately so the next q-block's PV never waits
                # on the reciprocal DMA chain
                for h in range(NH):
                    pv = pvs[h]
                    m = npl.tile([VW, 512], dt.float32, tag="m", name=f"m_{b}_{qb}_{h}")
                    nc.vector.tensor_copy(m[:], pv[:])
                    d4 = npl.tile([128, 4], dt.float32, tag="d4", name=f"d4_{b}_{qb}_{h}")
                    nc.sync.dma_start(
                        d4[:],
                        m[HD:VW, :].rearrange("p (a c) -> p a c", c=4),
                    )
                    r4 = npl.tile([128, 4], dt.float32, tag="r4", name=f"r4_{b}_{qb}_{h}")
                    nc.vector.reciprocal(r4[:], d4[:])
                    rc = npl.tile([1, 512], dt.float32, tag="rc", name=f"rc_{b}_{qb}_{h}")
                    nc.sync.dma_start(
                        rc[:].rearrange("p (a c) -> p a c", c=4), r4[:]
                    )
                    rbt = npl.tile([HD, 512], dt.float32, tag="rb", name=f"rb_{b}_{qb}_{h}")
                    nc.sync.dma_start(
                        rbt[:],
                        rc[0:1, :].rearrange("p (o q) -> p o q", o=1)
                        .broadcast_to((1, HD, 512)),
                    )
                    osl = ot_sb[h * HD:(h + 1) * HD, q0:q0 + 512]
                    nc.vector.tensor_mul(osl, m[0:HD, :], rbt[:])
                    nc.vector.tensor_scalar_add(
                        osl, osl, bv_sb[h * HD:(h + 1) * HD, :]
                    )
                fill(100)   # drain any leftover fillers

            # ---------------- emission schedule ----------------
            # batch 0 projections upfront
            for rb in range(4):
                for f in p1_block_fillers(rb):
                    f()
            for b in range(B):
                for qb in range(NQB):
                    fillers = []
                    if b < B - 1:
                        fillers.extend(p1_block_fillers((b + 1) * 4 + qb))
                    if b > 0:
                        for g in range((b - 1) * 16 + qb * 4,
                                       (b - 1) * 16 + qb * 4 + 4):
                            fillers.append(lambda g=g: p3_tile(g))
                    p2_qblock(b, qb, fillers)
            # tail: output projection for batch 3
            for g in range(48, 64):
                p3_tile(g)
    return nc


_NC_CACHE = None


def _get_nc():
    global _NC_CACHE
    if _NC_CACHE is None:
        _NC_CACHE = build_bass()
    return _NC_CACHE


def make_in_maps(x, Wq, bq, Wk, bk, Wv, bv, Wo, bo):
    xt = np.ascontiguousarray(
        np.asarray(x, dtype=np.float32).reshape(R, EMB).astype(bf16).T
    )
    in_maps = []
    for c in range(NCORES):
        rows = slice(F * c, F * (c + 1))
        in_maps.append({
            "xt": xt,
            "wq": np.ascontiguousarray(np.asarray(Wq)[rows, :].T.astype(bf16)),
            "wk": np.ascontiguousarray(np.asarray(Wk)[rows, :].T.astype(bf16)),
            "wv": np.ascontiguousarray(np.asarray(Wv)[rows, :].T.astype(bf16)),
            "wo": np.ascontiguousarray(np.asarray(Wo)[:, rows].T.astype(bf16)),
            "bq": np.asarray(bq)[rows].reshape(F, 1).astype(np.float32),
            "bv": np.asarray(bv)[rows].reshape(F, 1).astype(np.float32),
        })
    return in_maps


def gather(results, bo):
    acc = np.zeros((R, EMB), np.float32)
    for r in results:
        acc += r["out"].astype(np.float32)
    acc += np.asarray(bo, dtype=np.float32)
    return acc.reshape(B, T, EMB)


def kernel(x, Wq, bq, Wk, bk, Wv, bv, Wo, bo, _trace=False):
    nc = _get_nc()
    in_maps = make_in_maps(x, Wq, bq, Wk, bk, Wv, bv, Wo, bo)
    res = run_bass_kernel_spmd(nc, in_maps, list(range(NCORES)), trace=_trace)
    out = gather(res.results, bo)
    if _trace:
        kernel.last_result = res
    return out
